# revision 6
# baseline (speedup 1.0000x reference)
"""RNN-T decoder + joint network Trainium2 kernel.

Strategy (8 NeuronCores, data-parallel over batch B=16 -> 2 per core):
  - Host: embedding gather (pure indexing) + layout transposes + bf16 casts.
  - Device, per core, all layouts feature-major (partition = feature):
      1. enc_pT = W_enc @ hsT + b_enc            (batched GEMM)
      2. xp0T   = W_ih0 @ eysT + (b_ih0+b_hh0)   (batched GEMM, hoisted
         out of the recurrence)
      3. 64 sequential LSTM steps, layer 0: only W_hh0 @ h0 in the loop
         (weight-stationary matmuls; gates feature-major so elementwise
         ops run on full 128-lane tiles)
      4. xp1T   = W_ih1 @ H0all + (b_ih1+b_hh1)  (batched GEMM)
      5. 64 sequential LSTM steps, layer 1 (W_hh1 only)
      6. dec_pT = W_dec @ H1all                  (batched GEMM)
      7. Joint: per (b, u-pair): zT = tanh(enc_pT + dec_pT[u]) fused on
         ScalarE (bias = per-partition dec_p column); W_out-stationary
         matmuls stream zT (N=400); bias-add in-place on PSUM (VectorE);
         DMA straight from PSUM into a per-core [b][o][u][t] layout
         (800B-contiguous runs).  Host returns a transposed copy.

  Sync-wait discipline: this walrus allows ONE sync wait per ACT/DVE
  instruction (matmul allows more).  Hence: all GEMM readouts/bias adds
  are on VectorE (so the LSTM pre-add's xp input is same-engine), bias
  tiles are staged through a VectorE copy, LSTM h is written straight
  into its per-step column (no WAR on state), and the joint output is
  biased in-place in PSUM and DMA'd from PSUM (no SBUF staging tile).
"""

import os
import sys

import numpy as np

sys.path.insert(0, "/opt/trn_rl_repo")

import ml_dtypes  # noqa: E402
import concourse.bass as bass  # noqa: E402
from concourse import bacc  # noqa: E402
import concourse.mybir as mybir  # noqa: E402
import concourse.tile as tile  # noqa: E402
from concourse.bass_utils import run_bass_kernel_spmd  # noqa: E402

F32 = mybir.dt.float32
BF16 = mybir.dt.bfloat16
AF = mybir.ActivationFunctionType
ALU = mybir.AluOpType
BF_NP = ml_dtypes.bfloat16

NCORES = 8
B = 2        # batch per core
T = 200
U = 64
E = 512      # encoder proj dim
D = 512      # decoder hidden
J = 512      # joint dim
ODIM = 600
KB = 4       # 512 // 128
GT = 16      # 2048 // 128 gate tiles
R = B * U    # 128 LSTM rows per core
RT = B * T   # 400 encoder rows per core
OMW = [128, 128, 128, 128, 88]  # output feature tiles (600)


def _build():
    nc = bacc.Bacc()

    hst = nc.dram_tensor("hst", [E, RT], BF16, kind="ExternalInput")
    eyst = nc.dram_tensor("eyst", [E, R], BF16, kind="ExternalInput")
    wih0t = nc.dram_tensor("wih0t", [E, 4 * D], BF16, kind="ExternalInput")
    whh0t = nc.dram_tensor("whh0t", [D, 4 * D], BF16, kind="ExternalInput")
    wih1t = nc.dram_tensor("wih1t", [D, 4 * D], BF16, kind="ExternalInput")
    whh1t = nc.dram_tensor("whh1t", [D, 4 * D], BF16, kind="ExternalInput")
    wenct = nc.dram_tensor("wenct", [E, J], BF16, kind="ExternalInput")
    wdect = nc.dram_tensor("wdect", [D, J], BF16, kind="ExternalInput")
    woutt = nc.dram_tensor("woutt", [J, ODIM], BF16, kind="ExternalInput")
    bias0 = nc.dram_tensor("bias0", [128, GT], F32, kind="ExternalInput")
    bias1 = nc.dram_tensor("bias1", [128, GT], F32, kind="ExternalInput")
    benc = nc.dram_tensor("benc", [128, KB], F32, kind="ExternalInput")
    bout = nc.dram_tensor("bout", [128, len(OMW)], F32, kind="ExternalInput")
    outt = nc.dram_tensor("outt", [B, ODIM, U, T], F32, kind="ExternalOutput")

    with tile.TileContext(nc) as tc:
        with (
            tc.tile_pool(name="const", bufs=1) as cp,
            tc.tile_pool(name="work", bufs=2) as wp,
            tc.tile_pool(name="zt", bufs=2) as zp,
            tc.tile_pool(name="osb", bufs=4) as obp,
            tc.tile_pool(name="ps", bufs=2, space="PSUM") as psp,
            tc.tile_pool(name="pj", bufs=5, space="PSUM") as pjp,
        ):
            def load_kt(dram, cols, name):
                ts_ = []
                for k in range(dram.shape[0] // 128):
                    t = cp.tile([128, cols], BF16, tag=f"{name}{k}")
                    nc.sync.dma_start(out=t[:], in_=dram[k * 128:(k + 1) * 128, :])
                    ts_.append(t)
                return ts_

            wih0_sb = load_kt(wih0t, 4 * D, "wih0")
            whh0_sb = load_kt(whh0t, 4 * D, "whh0")
            wih1_sb = load_kt(wih1t, 4 * D, "wih1")
            whh1_sb = load_kt(whh1t, 4 * D, "whh1")
            wenc_sb = load_kt(wenct, J, "wenc")
            wdec_sb = load_kt(wdect, J, "wdec")
            wout_sb = load_kt(woutt, ODIM, "wout")
            hst_sb = load_kt(hst, RT, "hst")
            eyst_sb = load_kt(eyst, R, "eyst")

            def load_bias(dram, cols, name):
                raw = cp.tile([128, cols], F32, tag=f"{name}_raw")
                nc.sync.dma_start(out=raw[:], in_=dram[:, :])
                stg = cp.tile([128, cols], F32, tag=name)
                nc.vector.tensor_copy(stg[:], raw[:])  # stage onto DVE
                return stg

            b0_sb = load_bias(bias0, GT, "b0")
            b1_sb = load_bias(bias1, GT, "b1")
            benc_sb = load_bias(benc, KB, "benc")
            bout_sb = load_bias(bout, len(OMW), "bout")

            # persistent state / intermediates
            c0 = cp.tile([128, KB * B], F32, tag="c0")
            c1 = cp.tile([128, KB * B], F32, tag="c1")
            h0all = cp.tile([128, KB * R], BF16, tag="h0all")  # col k*128+b*64+u
            h1all = cp.tile([128, KB * R], BF16, tag="h1all")
            xp0 = cp.tile([128, GT * R], F32, tag="xp0")  # col t*128+b*64+u
            xp1 = cp.tile([128, GT * R], F32, tag="xp1")
            decp = cp.tile([128, KB * R], F32, tag="decp")  # col m*128+b*64+u
            encp = cp.tile([128, KB * RT], F32, tag="encp")  # col m*400+b*200+t

            nc.vector.memset(c0[:], 0.0)
            nc.vector.memset(c1[:], 0.0)

            # ---- encoder projection: enc_pT = W_enc @ hsT + b_enc ----
            for m in range(KB):
                pe_ = psp.tile([128, RT], F32, tag="ps")
                for k in range(KB):
                    nc.tensor.matmul(
                        pe_[:], wenc_sb[k][:, m * 128:(m + 1) * 128], hst_sb[k][:],
                        start=(k == 0), stop=(k == KB - 1))
                nc.vector.tensor_scalar_add(
                    encp[:, m * RT:(m + 1) * RT], pe_[:], benc_sb[:, m:m + 1])

            # ---- xp = W_ih @ rhs + bias (batched input projections) ----
            def in_proj(w_sb, rhs_fn, bias_sb, dst):
                for t in range(GT):
                    pb = psp.tile([128, R], F32, tag="ps")
                    for k in range(KB):
                        nc.tensor.matmul(
                            pb[:], w_sb[k][:, t * 128:(t + 1) * 128], rhs_fn(k),
                            start=(k == 0), stop=(k == KB - 1))
                    nc.vector.tensor_scalar_add(
                        dst[:, t * R:(t + 1) * R], pb[:], bias_sb[:, t:t + 1])

            in_proj(wih0_sb, lambda k: eyst_sb[k][:], b0_sb, xp0)

            def lstm_step(u, xp, whh_sb, cst, hall):
                # gates^T = W_hh @ h[u-1] + xp[:, u]; feature-major [128, t*B+b]
                hav = hall[:].rearrange("p (k b u) -> p k b u", k=KB, b=B)
                xpv = xp[:].rearrange("p (t b u) -> p t b u", t=GT, b=B)
                pre = wp.tile([128, GT * B], F32, tag="pre")
                prv = pre[:].rearrange("p (t b) -> p t b", t=GT)
                if u == 0:
                    nc.vector.tensor_copy(prv[:], xpv[:, :, :, 0])
                else:
                    pg = psp.tile([128, GT * B], F32, tag="ps")
                    for t in range(GT):
                        for k in range(KB):
                            nc.tensor.matmul(
                                pg[:, t * B:(t + 1) * B],
                                whh_sb[k][:, t * 128:(t + 1) * 128],
                                hav[:, k, :, u - 1],
                                start=(k == 0), stop=(k == KB - 1))
                    nc.vector.tensor_tensor(
                        prv[:], pg[:].rearrange("p (t b) -> p t b", t=GT),
                        xpv[:, :, :, u], ALU.add)
                ga = wp.tile([128, GT * B], F32, tag="ga")
                s = KB * B  # 8 cols per gate
                for q, fn in enumerate([AF.Sigmoid, AF.Sigmoid, AF.Tanh, AF.Sigmoid]):
                    nc.scalar.activation(
                        ga[:, q * s:(q + 1) * s], pre[:, q * s:(q + 1) * s],
                        fn, bias=0.0, scale=1.0)
                t2 = wp.tile([128, s], F32, tag="t2")
                nc.vector.tensor_tensor(t2[:], ga[:, 0:s], ga[:, 2 * s:3 * s], ALU.mult)
                t1 = wp.tile([128, s], F32, tag="t1")
                nc.vector.tensor_tensor(t1[:], ga[:, s:2 * s], cst[:], ALU.mult)
                nc.vector.tensor_tensor(cst[:], t1[:], t2[:], ALU.add)
                tch = wp.tile([128, s], F32, tag="tch")
                nc.scalar.activation(tch[:], cst[:], AF.Tanh, bias=0.0, scale=1.0)
                # h (bf16) written straight into its per-step column
                nc.vector.tensor_tensor(
                    hav[:, :, :, u], ga[:, 3 * s:4 * s].rearrange(
                        "p (k b) -> p k b", k=KB),
                    tch[:].rearrange("p (k b) -> p k b", k=KB), ALU.mult)

            for u in range(U):
                lstm_step(u, xp0, whh0_sb, c0, h0all)

            in_proj(wih1_sb, lambda k: h0all[:, k * R:(k + 1) * R], b1_sb, xp1)

            for u in range(U):
                lstm_step(u, xp1, whh1_sb, c1, h1all)

            # ---- dec_pT = W_dec @ H1all ----
            for m in range(KB):
                pb = psp.tile([128, R], F32, tag="ps")
                for k in range(KB):
                    nc.tensor.matmul(
                        pb[:], wdec_sb[k][:, m * 128:(m + 1) * 128],
                        h1all[:, k * R:(k + 1) * R],
                        start=(k == 0), stop=(k == KB - 1))
                nc.vector.tensor_copy(decp[:, m * R:(m + 1) * R], pb[:])

            # ---- joint ----
            for b in range(B):
                for up in range(U // 2):
                    u0 = 2 * up
                    zt = zp.tile([128, KB * 2 * T], BF16, tag="zt")
                    for k in range(KB):
                        for uu in range(2):
                            c = decp[:, k * R + b * U + u0 + uu:
                                     k * R + b * U + u0 + uu + 1]
                            nc.scalar.activation(
                                zt[:, k * 2 * T + uu * T:k * 2 * T + (uu + 1) * T],
                                encp[:, k * RT + b * T:k * RT + (b + 1) * T],
                                AF.Tanh, bias=c, scale=1.0)
                    for m in range(len(OMW)):
                        mw = OMW[m]
                        pj = pjp.tile([128, 2 * T], F32, tag="pj")
                        for k in range(KB):
                            nc.tensor.matmul(
                                pj[0:mw, :],
                                wout_sb[k][:, m * 128:m * 128 + mw],
                                zt[:, k * 2 * T:(k + 1) * 2 * T],
                                start=(k == 0), stop=(k == KB - 1))
                        ob = obp.tile([128, 2 * T], F32, tag="ob")
                        # memset absorbs the slot's DMA-read WAR so the
                        # bias-add below only waits on PE (1-wait limit)
                        nc.vector.memset(ob[:], 0.0)
                        nc.vector.tensor_scalar_add(
                            ob[0:mw, :], pj[0:mw, :], bout_sb[0:mw, m:m + 1])
                        nc.sync.dma_start(
                            out=outt[b, m * 128:m * 128 + mw, u0:u0 + 2, :],
                            in_=ob[0:mw, :].rearrange("p (u t) -> p u t", u=2))
    return nc


_CACHE = {}


def _prep_host(inputs):
    f32 = np.float32
    hs = np.asarray(inputs["hs_pad"], f32)
    ys = np.asarray(inputs["ys_in_pad"]).astype(np.int64)
    emb = np.asarray(inputs["embed_table"], f32)
    eys = emb[ys]  # (16, 64, 512)

    def bt(x):  # transpose + bf16
        return np.ascontiguousarray(np.asarray(x, f32).T).astype(BF_NP)

    shared = {
        "wih0t": bt(inputs["W_ih0"]),
        "whh0t": bt(inputs["W_hh0"]),
        "wih1t": bt(inputs["W_ih1"]),
        "whh1t": bt(inputs["W_hh1"]),
        "wenct": bt(inputs["W_enc"]),
        "wdect": bt(inputs["W_dec"]),
        "woutt": bt(inputs["W_out"]),
        "bias0": np.ascontiguousarray(
            (np.asarray(inputs["b_ih0"], f32) + np.asarray(inputs["b_hh0"], f32))
            .reshape(GT, 128).T),
        "bias1": np.ascontiguousarray(
            (np.asarray(inputs["b_ih1"], f32) + np.asarray(inputs["b_hh1"], f32))
            .reshape(GT, 128).T),
        "benc": np.ascontiguousarray(
            np.asarray(inputs["b_enc"], f32).reshape(KB, 128).T),
    }
    bo = np.zeros(len(OMW) * 128, f32)
    bo[:ODIM] = np.asarray(inputs["b_out"], f32)
    shared["bout"] = np.ascontiguousarray(bo.reshape(len(OMW), 128).T)

    in_maps = []
    for c in range(NCORES):
        m = dict(shared)
        m["hst"] = np.ascontiguousarray(
            hs[B * c:B * (c + 1)].reshape(RT, E).T).astype(BF_NP)
        m["eyst"] = np.ascontiguousarray(
            eys[B * c:B * (c + 1)].reshape(R, E).T).astype(BF_NP)
        in_maps.append(m)
    return in_maps


def kernel(**inputs):
    if "nc" not in _CACHE:
        nc_ = _build()
        if not nc_.is_finalized():
            nc_.finalize()
        _CACHE["nc"] = nc_
    nc = _CACHE["nc"]
    in_maps = _prep_host(inputs)
    trace = bool(int(os.environ.get("KERNEL_TRACE", "0")))
    res = run_bass_kernel_spmd(nc, in_maps, list(range(NCORES)), trace=trace)
    _CACHE["last"] = res
    out = np.empty((NCORES * B, T, U, ODIM), np.float32)
    for c in range(NCORES):
        oc = res.results[c]["outt"]  # (B, 600, 64, 200)
        out[B * c:B * (c + 1)] = np.transpose(oc, (0, 3, 2, 1))
    return out
